# revision 5
# baseline (speedup 1.0000x reference)
"""DBRX-style MoE (E=16, top-4, C=2048, H=3584, N=1024 tokens) on 8 TRN2 cores.

Strategy (expert-parallel with 2D (h-chunk x token) slot packing):
  - Host: gating in fp64 (logits -> top-4 -> softmax weights). fp64 makes the
    selected expert SET maximally robust against fp rounding.
  - The SPMD program is a sequence of SLOTS, each with a fixed token width
    cap_j and h-chunk depth h_j; every core runs one CELL per slot. A cell
    computes h_j ffn h-chunks of one expert for all of that expert's tokens
    (up/gate -> silu glu -> partial down). Each expert's 28 h-chunks are
    spread over cells on different cores; partial y outputs are summed on
    the host. A small host-side search picks slot shapes minimizing
    sum(cap_j*h_j) subject to 8 cells/slot (fallback: expert-halves rank-
    blocked into 4 uniform slots). This cuts the SPMD padding tax vs
    whole-expert slots while weight DMA stays exactly one pass over the
    expert weights.
  - Device (per core, per cell): uT/gT = Wup/Wg @ xT (PSUM-accumulated over
    C chunks, f16 matmuls), hT = silu(gT) * uT * gate_weight, then
    yT = Wdown_slice @ hT accumulated over the cell's h-chunks. Partial
    outputs are staged in SBUF and written f16 (partials are summed on the
    host in fp32, well within the error budget).

DMA layouts keep per-row transfers fat (>= 7KB for the weight streams): the
DMA engines process a roughly fixed row rate, so thin rows starve the PE.
W_up/W_gate pair into one 8KB-row transfer per h-chunk; W_down c-tiles group
into ~7KB-row transfers; y is staged and written as two fat-row transfers.
A short warm-up of matmuls on zeroed data runs during the initial input DMAs
so the PE HAM clock-gate is already at 8/8 when real work starts.

Cells padded beyond their expert's h-chunks carry zero weights (contribute
exactly 0); token columns beyond the expert's count have gate weight 0 and
are never read back.
"""

import math

import numpy as np

E, TOPK = 16, 4
C, H = 2048, 3584
B, T = 2, 512
N = B * T
N_CORES = 8
C_CHUNKS = C // 128  # 16
H_CHUNKS = H // 128  # 28
WARMUP_MMS = 45

_NC_CACHE: dict[tuple, object] = {}


def _pad4(v: int) -> int:
    return max(64, int(math.ceil(v / 4)) * 4)


def _wd_group(h: int) -> int:
    """c-tiles per W_down DMA: keep rows ~7KB; must divide C_CHUNKS."""
    g = 1
    while g < 16 and g * h * 256 < 7168:
        g *= 2
    return g


def _search_slots(counts):
    """Pick slot shapes (w_j, h_j) minimizing sum(w*h) with 8 cells/slot.

    Returns (slots, cells) where slots = [(w, h, g)], cells[j][k] =
    (expert, chunk_lo, n_chunks) or None. Falls back to expert-half
    rank-blocking if the search finds nothing feasible.
    """
    import itertools as it

    def emit(ws, hs):
        S = len(ws)
        rem = [H_CHUNKS] * E
        cur = [0] * E
        cells = []
        for j in range(S):
            w, h = ws[j], hs[j]
            nxt = ws[j + 1] if j + 1 < S else -1
            slot_cells = []
            for _ in range(8):
                must = [i for i in range(E)
                        if counts[i] > nxt and rem[i] > 0 and counts[i] <= w]
                pool = must if must else [
                    i for i in range(E) if rem[i] > 0 and counts[i] <= w]
                if not pool:
                    slot_cells.append(None)
                    continue
                i = max(pool, key=lambda i: rem[i])
                take = min(h, rem[i])
                slot_cells.append((i, cur[i], take))
                cur[i] += take
                rem[i] -= take
            for i in range(E):
                if counts[i] > nxt and counts[i] <= w and rem[i] > 0:
                    return None  # must-drain violated
            cells.append(slot_cells)
        if any(rem):
            return None
        return cells

    best = None
    vals = sorted(set(counts), reverse=True)
    for S in (4, 5, 6):
        for ws in it.combinations(vals, S):
            ws = tuple(sorted(ws, reverse=True))
            if ws[0] < max(counts):
                continue
            musts = []
            for j in range(S):
                nxt = ws[j + 1] if j + 1 < S else -1
                musts.append(sum(H_CHUNKS for i in range(E)
                                 if ws[j] >= counts[i] > nxt))
            pads = [_pad4(w) for w in ws]
            h0s = [min(H_CHUNKS, max(2, -(-m // 8))) for m in musts]
            if best is not None and sum(p * h for p, h in zip(pads, h0s)) >= best[0]:
                continue
            for deltas in it.product((0, 1, 2), repeat=S):
                hs = tuple(
                    min(H_CHUNKS, max(2, -(-musts[j] // 8)) + deltas[j])
                    for j in range(S)
                )
                cost = sum(_pad4(w) * h for w, h in zip(ws, hs))
                if best is not None and cost >= best[0]:
                    continue
                cells = emit(ws, hs)
                if cells is not None:
                    best = (cost, ws, hs, cells)
    if best is None:
        # fallback: expert halves, rank blocks of 8 (uniform h=14 slots)
        order = sorted(
            ((e, hf) for e in range(E) for hf in (0, 1)),
            key=lambda u: -counts[u[0]],
        )
        hh = H_CHUNKS // 2
        ws, hs, cells = [], [], []
        for j in range(4):
            blk = order[j * 8:(j + 1) * 8]
            ws.append(max(counts[e] for e, _ in blk))
            hs.append(hh)
            cells.append([(e, hf * hh, hh) for e, hf in blk])
        best = (0, tuple(ws), tuple(hs), cells)
    _, ws, hs, cells = best
    # order slots by descending work so the kernel tail is small
    order = sorted(range(len(ws)), key=lambda j: -_pad4(ws[j]) * hs[j])
    slots = [(_pad4(ws[j]), hs[j], _wd_group(hs[j])) for j in order]
    cells = [cells[j] for j in order]
    return slots, cells


def _build_nc(cfg: tuple):
    import concourse.bacc as bacc
    import concourse.mybir as mybir
    import concourse.tile as tile

    f32 = mybir.dt.float32
    f16 = mybir.dt.float16
    S = len(cfg)

    nc = bacc.Bacc("TRN2", target_bir_lowering=False, debug=False)
    xgs, wbs, wugs, wds, yts = [], [], [], [], []
    for j, (cap, h, g) in enumerate(cfg):
        xgs.append(nc.dram_tensor(
            f"xg{j}", [128, C_CHUNKS * cap], f16, kind="ExternalInput"))
        wbs.append(nc.dram_tensor(
            f"wb{j}", [128, cap], f32, kind="ExternalInput"))
        wugs.append(nc.dram_tensor(
            f"wug{j}", [h, 128, 2 * C_CHUNKS * 128], f16, kind="ExternalInput"))
        wds.append(nc.dram_tensor(
            f"wd{j}", [C_CHUNKS // g, 128, g * h * 128], f16,
            kind="ExternalInput"))
        yts.append(nc.dram_tensor(
            f"yt{j}", [128, C_CHUNKS * cap], f16, kind="ExternalOutput"))

    with tile.TileContext(nc) as tc:
        with (
            tc.tile_pool(name="warm", bufs=1) as wmp,
            tc.tile_pool(name="xp", bufs=2) as xp,
            tc.tile_pool(name="wp", bufs=6) as wp,
            tc.tile_pool(name="hp", bufs=2) as hp,
            tc.tile_pool(name="wdp", bufs=5) as wdp,
            tc.tile_pool(name="ysp", bufs=2) as ysp,
            tc.tile_pool(name="sp", bufs=3) as sp,
            tc.tile_pool(name="psw", bufs=1, space="PSUM") as psw,
            tc.tile_pool(name="psu", bufs=2, space="PSUM") as psu,
            tc.tile_pool(name="psg", bufs=2, space="PSUM") as psg,
            tc.tile_pool(name="psy", bufs=2, space="PSUM") as psy,
        ):
            goff = C_CHUNKS * 128
            # first slot: u-half weight DMA first so matmul starts ASAP
            cap0 = cfg[0][0]
            wt0 = wp.tile([128, 2 * C_CHUNKS * 128], f16, tag="wug")
            nc.sync.dma_start(wt0[:, :goff], wugs[0].ap()[0][:, :goff])
            xt0 = xp.tile([128, C_CHUNKS * cap0], f16, tag="xg")
            for c0, g_ in ((0, 4), (4, 6), (10, 6)):
                nc.sync.dma_start(
                    xt0[:, c0 * cap0:(c0 + g_) * cap0],
                    xgs[0].ap()[:, c0 * cap0:(c0 + g_) * cap0],
                )
            nc.sync.dma_start(wt0[:, goff:], wugs[0].ap()[0][:, goff:])

            # HAM warm-up on zeroed data (near-zero switching power) while
            # the first input DMAs stream
            if WARMUP_MMS:
                wmt = wmp.tile([128, 128], f16, tag="warm")
                nc.vector.memset(wmt[:], 0.0)
                wps = psw.tile([128, 128], f32, tag="warmps")
                for i in range(WARMUP_MMS):
                    nc.tensor.matmul(
                        wps[:], wmt[:], wmt[:],
                        start=(i == 0), stop=(i == WARMUP_MMS - 1),
                    )

            for j, (cap, hch, g) in enumerate(cfg):
                if j == 0:
                    xt = xt0
                else:
                    xt = xp.tile([128, C_CHUNKS * cap], f16, tag="xg")
                    nc.sync.dma_start(xt[:], xgs[j].ap())
                wbt = xp.tile([128, cap], f32, tag="wb")
                nc.sync.dma_start(wbt[:], wbs[j].ap())
                ht = hp.tile([128, hch * cap], f16, tag="ht")

                for h in range(hch):
                    if j == 0 and h == 0:
                        wt = wt0
                    else:
                        wt = wp.tile([128, 2 * C_CHUNKS * 128], f16, tag="wug")
                        nc.sync.dma_start(wt[:], wugs[j].ap()[h])
                    ups = psu.tile([128, cap], f32, tag="u")
                    gps = psg.tile([128, cap], f32, tag="g")
                    for c in range(C_CHUNKS):
                        nc.tensor.matmul(
                            ups[:],
                            wt[:, c * 128:(c + 1) * 128],
                            xt[:, c * cap:(c + 1) * cap],
                            start=(c == 0),
                            stop=(c == C_CHUNKS - 1),
                        )
                    for c in range(C_CHUNKS):
                        nc.tensor.matmul(
                            gps[:],
                            wt[:, goff + c * 128:goff + (c + 1) * 128],
                            xt[:, c * cap:(c + 1) * cap],
                            start=(c == 0),
                            stop=(c == C_CHUNKS - 1),
                        )
                    sg = sp.tile([128, cap], f32, tag="sg")
                    nc.scalar.activation(
                        sg[:], gps[:], mybir.ActivationFunctionType.Silu
                    )
                    uw = sp.tile([128, cap], f32, tag="uw")
                    nc.vector.tensor_mul(uw[:], ups[:], wbt[:])
                    nc.vector.tensor_mul(
                        ht[:, h * cap:(h + 1) * cap], sg[:], uw[:]
                    )

                stage = ysp.tile([128, C_CHUNKS * cap], f16, tag="yst")
                half = (C_CHUNKS // 2) * cap
                for grp in range(C_CHUNKS // g):
                    wdt = wdp.tile([128, g * hch * 128], f16, tag="wd")
                    nc.sync.dma_start(wdt[:], wds[j].ap()[grp])
                    for k in range(g):
                        ct = grp * g + k
                        koff = k * hch * 128
                        yps = psy.tile([128, cap], f32, tag="y")
                        for h in range(hch):
                            nc.tensor.matmul(
                                yps[:],
                                wdt[:, koff + h * 128:koff + (h + 1) * 128],
                                ht[:, h * cap:(h + 1) * cap],
                                start=(h == 0),
                                stop=(h == hch - 1),
                            )
                        nc.vector.tensor_copy(
                            stage[:, ct * cap:(ct + 1) * cap], yps[:]
                        )
                        if ct == C_CHUNKS // 2 - 1:
                            # first half staged: write it out now so only the
                            # second half remains after the last matmul
                            nc.sync.dma_start(
                                yts[j].ap()[:, :half], stage[:, :half]
                            )
                nc.sync.dma_start(yts[j].ap()[:, half:], stage[:, half:])
    nc.compile()
    return nc


def _get_nc(cfg: tuple):
    if cfg not in _NC_CACHE:
        _NC_CACHE[cfg] = _build_nc(cfg)
    return _NC_CACHE[cfg]


def _route(xf: np.ndarray, gate_inp: np.ndarray):
    """Host gating in fp64: per-expert token index lists + combine weights."""
    logits = xf.astype(np.float64) @ gate_inp.astype(np.float64).T  # [N, E]
    # top-4 (descending); fp64 makes ordering robust vs the fp32 reference
    topi = np.argsort(-logits, axis=1, kind="stable")[:, :TOPK]  # [N, K]
    topv = np.take_along_axis(logits, topi, axis=1)
    w = np.exp(topv - topv[:, :1])
    w /= w.sum(axis=1, keepdims=True)  # [N, K] fp64 softmax
    idxs, wts = [], []
    for e in range(E):
        sel = topi == e  # [N, K]
        rows = np.nonzero(sel.any(axis=1))[0]
        k_of_row = np.argmax(sel[rows], axis=1)  # which top-k slot holds e
        idxs.append(rows.astype(np.int64))
        wts.append(w[rows, k_of_row])
    return idxs, wts


def _prepare(x, W_up, W_gate, W_down, gate_inp):
    xf = np.ascontiguousarray(np.asarray(x, dtype=np.float32)).reshape(N, C)
    W_up = np.asarray(W_up, dtype=np.float32)
    W_gate = np.asarray(W_gate, dtype=np.float32)
    W_down = np.asarray(W_down, dtype=np.float32)
    gate_inp = np.asarray(gate_inp, dtype=np.float32)

    idxs, wts = _route(xf, gate_inp)
    counts = [len(i) for i in idxs]
    slots, cells = _search_slots(counts)
    cfg = tuple(slots)

    # per-expert cached prep: gathered x (f16) and transposed weight forms
    xg_e = {}
    upt_e, gpt_e, wdt_e = {}, {}, {}
    used = sorted({cell[0] for sc in cells for cell in sc if cell})
    for e in used:
        if counts[e]:
            xg_e[e] = xf[idxs[e]].astype(np.float16)
        # [h_chunk, q(c_in), c_chunk, h_col] -> [28, 128, 16*128]
        upt_e[e] = np.ascontiguousarray(
            W_up[e].reshape(H_CHUNKS, 128, C_CHUNKS, 128).transpose(0, 3, 2, 1)
        ).reshape(H_CHUNKS, 128, C_CHUNKS * 128).astype(np.float16)
        gpt_e[e] = np.ascontiguousarray(
            W_gate[e].reshape(H_CHUNKS, 128, C_CHUNKS, 128).transpose(0, 3, 2, 1)
        ).reshape(H_CHUNKS, 128, C_CHUNKS * 128).astype(np.float16)
        # [c_tile, q(h_in), h_chunk, c_col]
        wdt_e[e] = np.ascontiguousarray(
            W_down[e].reshape(C_CHUNKS, 128, H_CHUNKS, 128).transpose(0, 3, 2, 1)
        ).astype(np.float16)

    in_maps = []
    for core in range(N_CORES):
        im = {}
        for j, (cap, hch, g) in enumerate(slots):
            cell = cells[j][core]
            wug = np.zeros((hch, 128, 2 * C_CHUNKS * 128), np.float16)
            wd = np.zeros((C_CHUNKS // g, 128, g * hch * 128), np.float16)
            xgj = np.zeros((128, C_CHUNKS * cap), np.float16)
            wb = np.zeros((128, cap), np.float32)
            if cell is not None:
                e, lo, n = cell
                idx, wvec = idxs[e], wts[e]
                cnt = len(idx)
                xge = np.zeros((cap, C), np.float16)
                if cnt:
                    xge[:cnt] = xg_e[e]
                # [q, c_chunk, t] <- xge[t, c_chunk*128+q]
                xgj[:] = np.ascontiguousarray(
                    xge.reshape(cap, C_CHUNKS, 128).transpose(2, 1, 0)
                ).reshape(128, C_CHUNKS * cap)
                wb[:, :cnt] = np.float32(wvec)[None, :]
                wug[:n, :, :C_CHUNKS * 128] = upt_e[e][lo:lo + n]
                wug[:n, :, C_CHUNKS * 128:] = gpt_e[e][lo:lo + n]
                # wd[grp, :, (k*hch + h)*128 : ...] = W_down c-tile grp*g+k,
                # h-chunk lo+h
                wdu = wdt_e[e][:, :, lo:lo + n, :]  # [16, 128, n, 128]
                wdr = wd.reshape(C_CHUNKS // g, 128, g, hch, 128)
                wdr[:, :, :, :n] = wdu.reshape(
                    C_CHUNKS // g, g, 128, n, 128).transpose(0, 2, 1, 3, 4)
            im[f"xg{j}"] = xgj
            im[f"wb{j}"] = wb
            im[f"wug{j}"] = wug
            im[f"wd{j}"] = wd
        in_maps.append(im)
    return in_maps, cfg, cells, idxs


def _combine(results, cfg, cells, idxs):
    y = np.zeros((N, C), np.float32)
    for core in range(N_CORES):
        for j, (cap, _hch, _g) in enumerate(cfg):
            cell = cells[j][core]
            if cell is None:
                continue
            e, _lo, _n = cell
            idx = idxs[e]
            cnt = len(idx)
            if not cnt:
                continue
            # yt [128, 16*cap]: value at (p, ct*cap + t) = y[token t, ct*128+p]
            ytf = (
                results[core][f"yt{j}"]
                .reshape(128, C_CHUNKS, cap)
                .transpose(1, 0, 2)
                .reshape(C, cap)
            )
            y[idx] += ytf[:, :cnt].T.astype(np.float32)
    return y.reshape(B, T, C)


def kernel(x, W_up, W_gate, W_down, gate_inp):
    from concourse import bass_utils

    in_maps, cfg, cells, idxs = _prepare(x, W_up, W_gate, W_down, gate_inp)
    nc = _get_nc(cfg)
    res = bass_utils.run_bass_kernel_spmd(nc, in_maps, core_ids=list(range(N_CORES)))
    kernel.last_result = res
    return _combine(res.results, cfg, cells, idxs)
